# revision 4
# baseline (speedup 1.0000x reference)
"""ArcFace fully-connected loss head on 8 Trainium2 NeuronCores.

Computes  out = s * (onehot(label) * phi + (1-onehot) * cos)  where
cos = l2norm(x) @ l2norm(W).T, phi = cos(arccos(cos)+m) with the ArcFace
threshold branch.

Distribution: classification-parallel (Partial-FC style). The class dim
C=100000 is split into 8 contiguous shards of 12500; every core gets the
full input x (replicated) and its weight shard, and produces its
[512, 12500] slice of the output. No collectives needed.

Device kernel per core:
  - normalize x rows (ACT square+accum -> sqrt -> reciprocal -> one
    Newton step), cast to bf16, PE-transpose to get x^T [D,B] stationary.
  - stream weight shard in chunks of 125 rows: ACT square+accum gives
    row norms in the same pass; scale rows by 1/||w|| on DVE (cast bf16);
    PE-transpose to [D, C-chunk]; matmul accumulating over D in PSUM;
    evacuate PSUM via ACT copy with scale=s; DMA out.
  - the ArcFace margin only changes the single label column per row
    (512 of 51.2M elements), so the host applies it to the returned
    s*cos values; the device emits s*cos everywhere.
"""

import math
import sys

sys.path.insert(0, "/opt/trn_rl_repo")

import numpy as np

B, D, C = 512, 512, 100000
N_CORES = 8
CL = C // N_CORES  # 12500 classes per core
S_SCALE = 30.0
MARGIN = 0.5
COS_M = math.cos(MARGIN)
SIN_M = math.sin(MARGIN)
TH = math.cos(math.pi - MARGIN)
MM = math.sin(math.pi - MARGIN) * MARGIN

# tiling
CCHUNK = 125           # weight rows per natural chunk (partition dim)
NJ = 4                 # natural chunks per super-chunk
SC = CCHUNK * NJ       # 500 classes per super-chunk (matmul N)
NSC = CL // SC         # 25 super-chunks per core
KD = D // 128          # 4 contraction chunks
NB = B // 128          # 4 batch chunks

_CACHE = {}


def _rsqrt(nc, mybir, pool, x_ap, p, n):
    """y ~= 1/sqrt(x) for a small [p, n] f32 AP, refined by one Newton step.

    ACT's Sqrt LUT has a loose precision budget, so refine
    y0 = 1/sqrt_act(x) with y1 = y0*(1.5 - 0.5*x*y0^2) on DVE.
    """
    AF = mybir.ActivationFunctionType
    ALU = mybir.AluOpType
    f32 = mybir.dt.float32
    nrm = pool.tile([128, n], f32, tag="rs_nrm")
    nc.scalar.activation(out=nrm[:p], in_=x_ap, func=AF.Sqrt)
    y0 = pool.tile([128, n], f32, tag="rs_y0")
    nc.vector.reciprocal(out=y0[:p], in_=nrm[:p])
    t = pool.tile([128, n], f32, tag="rs_t")
    nc.vector.tensor_mul(t[:p], y0[:p], y0[:p])
    nc.vector.tensor_mul(t[:p], t[:p], x_ap)
    nc.vector.tensor_scalar(
        out=t[:p], in0=t[:p], scalar1=-0.5, scalar2=1.5, op0=ALU.mult, op1=ALU.add
    )
    nc.vector.tensor_mul(t[:p], t[:p], y0[:p])
    return t


def _build():
    if "nc" in _CACHE:
        return _CACHE["nc"]
    from contextlib import ExitStack

    import concourse.mybir as mybir
    import concourse.tile as tile
    from concourse import bacc
    from concourse.masks import make_identity

    f32 = mybir.dt.float32
    bf16 = mybir.dt.bfloat16
    AF = mybir.ActivationFunctionType

    nc = bacc.Bacc("TRN2", target_bir_lowering=False)
    x_d = nc.dram_tensor("input", [B, D], f32, kind="ExternalInput")
    w_d = nc.dram_tensor("weight", [CL, D], f32, kind="ExternalInput")
    o_d = nc.dram_tensor("out", [B, CL], f32, kind="ExternalOutput")

    with tile.TileContext(nc) as tc, ExitStack() as ctx:
        singles = ctx.enter_context(tc.tile_pool(name="singles", bufs=1))
        stats = ctx.enter_context(tc.tile_pool(name="stats", bufs=4))
        sqpool = ctx.enter_context(tc.tile_pool(name="sqpool", bufs=3))
        xpool = ctx.enter_context(tc.tile_pool(name="xpool", bufs=4))
        wpool = ctx.enter_context(tc.tile_pool(name="wpool", bufs=8))
        wnpool = ctx.enter_context(tc.tile_pool(name="wnpool", bufs=8))
        wntpool = ctx.enter_context(tc.tile_pool(name="wntpool", bufs=3))
        outpool = ctx.enter_context(tc.tile_pool(name="outpool", bufs=6))
        mmpsum = ctx.enter_context(tc.tile_pool(name="mmpsum", bufs=4, space="PSUM"))
        tpsum = ctx.enter_context(tc.tile_pool(name="tpsum", bufs=4, space="PSUM"))

        ident = singles.tile([128, 128], f32)
        make_identity(nc, ident)

        # ---- x: normalize rows, transpose to xnT[d_part, kd, b] ----
        xnT = singles.tile([128, KD, B], bf16)
        xn2 = singles.tile([128, NB], f32)
        xts = []
        for bi in range(NB):
            xt = xpool.tile([128, D], f32, tag="xt")
            nc.sync.dma_start(out=xt, in_=x_d[bi * 128 : (bi + 1) * 128, :])
            sq = sqpool.tile([128, D], bf16, tag="sq")
            nc.scalar.activation(
                out=sq, in_=xt, func=AF.Square, accum_out=xn2[:, bi : bi + 1]
            )
            xts.append(xt)
        xrn = _rsqrt(nc, mybir, stats, xn2[:, :], 128, NB)
        for bi in range(NB):
            xnb = xpool.tile([128, D], f32, tag="xnb")
            nc.vector.tensor_scalar_mul(xnb, xts[bi], xrn[:, bi : bi + 1])
            for kd in range(KD):
                pst = tpsum.tile([128, SC], f32, tag="pst")
                nc.tensor.transpose(
                    pst[:, :128], xnb[:, kd * 128 : (kd + 1) * 128], ident
                )
                nc.vector.tensor_copy(
                    out=xnT[:, kd, bi * 128 : (bi + 1) * 128], in_=pst[:, :128]
                )

        # ---- stream weight shard ----
        for sc in range(NSC):
            c0 = sc * SC
            wn2 = stats.tile([128, NJ], f32, tag="wn2")
            wts = []
            for j in range(NJ):
                wt = wpool.tile([128, D], f32, tag="wt")
                nc.sync.dma_start(
                    out=wt[:CCHUNK, :],
                    in_=w_d[c0 + j * CCHUNK : c0 + (j + 1) * CCHUNK, :],
                )
                sq = sqpool.tile([128, D], bf16, tag="sq")
                nc.scalar.activation(
                    out=sq[:CCHUNK],
                    in_=wt[:CCHUNK],
                    func=AF.Square,
                    accum_out=wn2[:CCHUNK, j : j + 1],
                )
                wts.append(wt)
            rn = _rsqrt(nc, mybir, stats, wn2[:CCHUNK, :], CCHUNK, NJ)
            wnbs = []
            for j in range(NJ):
                wnb = wnpool.tile([128, D], f32, tag="wnb")
                nc.vector.tensor_scalar_mul(
                    wnb[:CCHUNK], wts[j][:CCHUNK], rn[:CCHUNK, j : j + 1]
                )
                wnbs.append(wnb)
            wnT = wntpool.tile([128, KD, SC], bf16, tag="wnT")
            for kd in range(KD):
                pst = tpsum.tile([128, SC], f32, tag="pst")
                for j in range(NJ):
                    nc.tensor.transpose(
                        pst[:, j * CCHUNK : (j + 1) * CCHUNK],
                        wnbs[j][:CCHUNK, kd * 128 : (kd + 1) * 128],
                        ident[:CCHUNK, :CCHUNK],
                    )
                nc.vector.tensor_copy(out=wnT[:, kd, :], in_=pst)
            for bi in range(NB):
                po = mmpsum.tile([128, SC], f32, tag="po")
                for kd in range(KD):
                    nc.tensor.matmul(
                        po,
                        xnT[:, kd, bi * 128 : (bi + 1) * 128],
                        wnT[:, kd, :],
                        start=(kd == 0),
                        stop=(kd == KD - 1),
                    )
                ot = outpool.tile([128, SC], f32, tag="ot")
                nc.scalar.activation(out=ot, in_=po, func=AF.Copy, scale=S_SCALE)
                nc.sync.dma_start(
                    out=o_d[bi * 128 : (bi + 1) * 128, c0 : c0 + SC], in_=ot
                )

    nc.compile()
    _CACHE["nc"] = nc
    return nc


def kernel(input, weight, label):
    from concourse.bass_utils import run_bass_kernel_spmd

    nc = _build()
    x = np.ascontiguousarray(np.asarray(input, dtype=np.float32))
    w = np.ascontiguousarray(np.asarray(weight, dtype=np.float32))
    in_maps = [
        {"input": x, "weight": w[k * CL : (k + 1) * CL]} for k in range(N_CORES)
    ]
    res = run_bass_kernel_spmd(nc, in_maps, core_ids=list(range(N_CORES)))
    out = np.concatenate([res.results[k]["out"] for k in range(N_CORES)], axis=1)

    # ArcFace margin on the label column of each row (device emitted s*cos)
    rows = np.arange(B)
    cols = np.asarray(label).astype(np.int64)
    cos = out[rows, cols].astype(np.float64) / S_SCALE
    sine = np.sqrt(np.maximum(0.0, 1.0 - cos * cos))
    phi = cos * COS_M - sine * SIN_M
    phi = np.where(cos > TH, phi, cos - MM)
    out[rows, cols] = (phi * S_SCALE).astype(np.float32)
    return out
